# revision 36
# baseline (speedup 1.0000x reference)
"""MoE layer (dense all-experts SwiGLU + router-weighted sum) on 8 TRN2 cores.

Expert-parallel: core e holds expert e's weights (E=8). Every core sees the
full token stream x (shipped pre-transposed as xT [H, N]) and computes
  y_e = softmax(x @ W_router)[:, e] * ((silu(x@Wg_e) * (x@Wu_e)) @ Wd_e)
The host sums the 8 per-expert outputs in float64.

All matmul operands are fp16: measured on TRN2, fp16 matmuls stream at the
full 2.4 GHz PE clock (216ns per 512-col matmul) while fp32r runs ~2.2 GHz
(233ns) and bf16 only ~2.0 GHz (259ns). fp16 quantization costs ~6e-4 rel
err vs the 2e-2 gate. PSUM accumulation stays fp32. 2-byte weights also fit
ALL of Wg/Wu/Wd resident in SBUF (96KB/partition), so there is no steady-
state weight streaming at all: per block only xt in (1MB, sync HWDGE ring,
3 blocks deep) and y out (1MB fp16, same ring, issued after the xt
prefetch; the last block is m-outer so the tail drains in ~2us).

Per-core program, per 512-token block:
  router: logits via 2 waves of 4 COLUMN-TILED matmuls (col group j streams
          k-chunks j/j+4 concurrently, ~0.45us for what 8 serial matmuls did
          in 1.7us), partials collapsed by one full-K matmul against a
          host-built group-sum selector; Exp on ACT, DVE 32x32 block-
          transposes to token-partition layout, then numer/denom as DVE
          free-dim reductions against [ones | one-hot(e)]; w = num/den.
          Partials issue MID-STAGE-1 (after i=14's u-step), where every
          ACT-tick dependency from the previous cycle has drained - this
          placement removed the last per-block PE stall; the collapse runs
          after stage-2's first m-group, copies long done.
  stage1: G/U [128i, 512tok] = Wg/Wu_chunk^T @ xT_chunk (8 K-chunks into
          PSUM), hT[i] = silu(G)*U -> SBUF fp16 ([I, tok] layout).
  stage2: Y[m] [128tok, 512h] accumulates 16 i-chunks, hT stationary and
          resident Wd slices moving; evict = DVE multiply by w.

Block 0 runs stage 1 k-OUTER in groups of 4 i-chunks (borrowing psy PSUM
banks) so compute chases the 256KB Wg half-chunk DMAs as they land instead
of waiting for all of Wg. All DMA rides the sync HWDGE ring: it wakes
faster than SWDGE (~10.5us vs ~13us to first matmul) and no other queue's
semaphore set gets allocated.

Scheduling notes (hard-won):
  - tile-pool slot allocation order must match consumption order or the
    schedule deadlocks; 24 xt chunk tiles (3 blocks) are live at once.
  - stage 2 is m-OUTER: both h-halves of a token subtile accumulate
    together, so each m's evict+DMA overlaps the next m's matmuls.
  - psgu=3/psy=5 PSUM split: 3 g/u banks decouple the g-start matmuls
    from silu completion latency; router partials share psy's rotation.
  - never read uninitialized SBUF (the exp tile is memset before the
    transposes): it crashes the device, silently.
"""
import numpy as np

import concourse.bass as bass
import concourse.mybir as mybir
import concourse.tile as tile
from concourse import bacc
from concourse.bass_utils import run_bass_kernel_spmd

P = 128
H, I, E = 1024, 2048, 8
N = 8192  # tokens = 4 * 2048
HK = H // P   # 8 contraction chunks over H
IK = I // P   # 16 chunks over I
TB = 512      # token block
NB = N // TB  # 16 blocks
NM = TB // P  # 4 token subtiles per block
NH = H // 512  # 2 output column halves
PF = 3        # xt prefetch depth (blocks)

F32 = mybir.dt.float32
DT16 = mybir.dt.float16
AF = mybir.ActivationFunctionType

# set by a driver (test.py) to profile; harness path keeps defaults
TRACE = False
LAST_EXEC_NS = None

_CACHE = {}


def _build():
    nc = bacc.Bacc("TRN2", target_bir_lowering=False, debug=False)

    xt_d = nc.dram_tensor("xt", [H, N], DT16, kind="ExternalInput").ap()
    wg_d = nc.dram_tensor("wg", [H, I], DT16, kind="ExternalInput").ap()
    wu_d = nc.dram_tensor("wu", [H, I], DT16, kind="ExternalInput").ap()
    wd_d = nc.dram_tensor("wd", [I, H], DT16, kind="ExternalInput").ap()
    wr_d = nc.dram_tensor("wr", [P, HK * E], DT16, kind="ExternalInput").ap()
    sel_d = nc.dram_tensor("sel", [P, 3 * E], DT16, kind="ExternalInput").ap()
    y_d = nc.dram_tensor("y", [N, H], DT16, kind="ExternalOutput").ap()

    with tile.TileContext(nc) as tc:
        with (
            tc.tile_pool(name="const", bufs=1) as const,
            tc.tile_pool(name="xtp", bufs=PF * HK) as xtp,
            tc.tile_pool(name="htp", bufs=1) as htp,
            tc.tile_pool(name="evp", bufs=8) as evp,
            tc.tile_pool(name="rtp", bufs=2) as rtp,
            tc.tile_pool(name="wp", bufs=2) as wp,
            tc.tile_pool(name="psgu", bufs=3, space="PSUM") as psgu,
            tc.tile_pool(name="psy", bufs=5, space="PSUM") as psy,
        ):
            # resident weights: [128, HK*I] with chunk k at cols [k*I, (k+1)*I)
            wg_sb = const.tile([P, HK * I], DT16)
            wu_sb = const.tile([P, HK * I], DT16)
            # wd resident: [128, IK*H] with i-chunk at cols [i*H, (i+1)*H)
            wd_sb = const.tile([P, IK * H], DT16)
            wr_sb = const.tile([P, HK * E], DT16)
            sel_sb = const.tile([P, 3 * E], DT16)

            def load_xt(b, eng=None):
                eng = eng or nc.sync
                tok = slice(b * TB, (b + 1) * TB)
                chunks = []
                for k in range(HK):
                    ch = xtp.tile([P, TB], DT16, tag="xt", name=f"xt{b}_{k}")
                    eng.dma_start(
                        out=ch[:], in_=xt_d[k * P:(k + 1) * P, tok]
                    )
                    chunks.append(ch)
                return chunks

            # ---- prologue DMAs in consumption order, all on the sync
            # ring (it wakes faster than SWDGE: first matmul 10.5us vs
            # 13.3us measured; and never touching SWDGE also skips its
            # semaphore set in the teardown).
            IH = I // 2
            nc.sync.dma_start(out=wr_sb[:], in_=wr_d[:])
            xt_next = load_xt(0)
            nc.sync.dma_start(out=sel_sb[:], in_=sel_d[:])
            # wg streamed as 256KB half-chunks, half-A (i 0-7) of every k
            # first: block-0's k-outer groups consume them as they land.
            for half in range(2):
                for k in range(HK):
                    nc.sync.dma_start(
                        out=wg_sb[:, k * I + half * IH: k * I + (half + 1) * IH],
                        in_=wg_d[k * P:(k + 1) * P, half * IH:(half + 1) * IH],
                    )
            xt_pre1 = load_xt(1)
            for half in range(2):
                for k in range(HK):
                    nc.sync.dma_start(
                        out=wu_sb[:, k * I + half * IH: k * I + (half + 1) * IH],
                        in_=wu_d[k * P:(k + 1) * P, half * IH:(half + 1) * IH],
                    )
            for j in range(IK // 2):  # wd as 512KB 2-chunk batches
                rows = slice(2 * j * P, (2 * j + 2) * P)
                nc.sync.dma_start(
                    out=wd_sb[:, 2 * j * H:(2 * j + 2) * H].rearrange(
                        "p (j c) -> p j c", j=2),
                    in_=wd_d[rows, :].rearrange("(j p) c -> p j c", p=P),
                )

            def router_partials(xt_ch):
                # w[tok] = softmax(logits)[:, e] for one block. Only the 8
                # lt matmuls touch the PE: exp'd logits move to token-
                # partition layout via DVE 32x32 block transposes, and
                # numer/denom come from DVE free-dim accumulations against
                # selbc ([ones | one-hot(e)] replicated per partition).
                # 4-way column-tiled logits: col group j streams k-chunks
                # j and j+4 concurrently into partitions 32j..32j+7 of one
                # PSUM bank (~2 wave-times instead of 8 serial matmuls).
                # Only the first matmul clears the bank's has_written bits;
                # the other groups' first writes land on cleared bits and
                # overwrite, wave 2 accumulates.
                p4s = [
                    psy.tile([P, TB], F32, tag="y", name=f"lt4_{j}")
                    for j in range(4)
                ]
                for w in range(2):
                    for j in range(4):
                        k = 4 * w + j
                        nc.tensor.matmul(
                            p4s[j][32 * j:32 * j + E, :],
                            (wr_sb[:, k * E:(k + 1) * E]),
                            (xt_ch[k][:]),
                            start=(w == 0),
                            stop=(w == 1),
                            tile_position=(0, 32 * j),
                            skip_group_check=True,
                        )
                # collapse the 4 partition-group partials with one full-K
                # matmul against the group-sum selector. p4 is memset first:
                # the PE streams all 128 partitions and must never read
                # uninitialized SBUF (zero rows also zero out the garbage
                # via the selector's zero rows).
                p4 = rtp.tile([P, TB], DT16, tag="p4", name="p4")
                nc.vector.memset(p4[:], 0.0)
                for j in range(4):
                    nc.scalar.activation(
                        p4[32 * j:32 * j + E, :], p4s[j][32 * j:32 * j + E, :],
                        AF.Copy,
                    )
                return p4

            def router_finish(p4):
                # runs between the h-sweeps: by then the partial copies have
                # drained, so the sum-matmul issues without a PE stall
                lt = psgu.tile([E, TB], F32, tag="gu", name="lt")
                nc.tensor.matmul(
                    lt[:], (sel_sb[:, 2 * E:3 * E]), (p4[:]),
                    start=True, stop=True,
                )
                # zero all 32 rows first (partition base must be 32-
                # aligned, and the transposes must not read uninitialized
                # SBUF), then exp overwrites rows 0-7.
                exp_sb = rtp.tile([32, TB], DT16, tag="exp", name="exp_sb")
                nc.vector.memset(exp_sb[:], 0.0)
                nc.scalar.activation(exp_sb[0:E, :], lt[:], AF.Exp)
                wv = wp.tile([P, NM], F32, tag="wv", name="wv")
                for m in range(NM):
                    et = rtp.tile([P, 32], DT16, tag="et", name="et", bufs=4)
                    for j in range(4):
                        nc.vector.transpose(
                            out=et[j * 32:(j + 1) * 32, 0:32],
                            in_=exp_sb[0:32, m * P + j * 32:
                                       m * P + (j + 1) * 32],
                        )
                    junk = wp.tile([P, E], F32, tag="junk", name="junk")
                    den = wp.tile([P, 1], F32, tag="den", name="den")
                    nc.vector.scalar_tensor_tensor(
                        out=junk[:], in0=et[:, 0:E], scalar=1.0,
                        in1=sel_sb[:, 0:E], op0=mybir.AluOpType.mult,
                        op1=mybir.AluOpType.mult, accum_out=den[:],
                    )
                    num = wp.tile([P, 1], F32, tag="num", name="num")
                    nc.vector.scalar_tensor_tensor(
                        out=junk[:], in0=et[:, 0:E], scalar=1.0,
                        in1=sel_sb[:, E:2 * E], op0=mybir.AluOpType.mult,
                        op1=mybir.AluOpType.mult, accum_out=num[:],
                    )
                    rec = wp.tile([P, 1], F32, tag="rec", name="rec")
                    nc.vector.reciprocal(rec[:], den[:])
                    nc.vector.tensor_tensor(
                        out=wv[:, m:m + 1], in0=num[:], in1=rec[:],
                        op=mybir.AluOpType.mult,
                    )
                return wv

            xtq = [xt_next, xt_pre1]  # blocks b, b+1 (already issued)
            w_next = router_finish(router_partials(xtq[0]))
            for b in range(NB):
                xt_ch = xtq.pop(0)
                w_tiles = w_next
                if b + 2 < NB:
                    xtq.append(load_xt(b + 2))

                # ---- stage 1: hT[i] = silu(G)*U, [I-chunk, tok] layout
                ht_sb = htp.tile([P, IK * TB], DT16, tag="ht")

                def g_step(i):
                    g_ps = psgu.tile([P, TB], F32, tag="gu", name="g_ps")
                    for k in range(HK):
                        nc.tensor.matmul(
                            g_ps[:],
                            (wg_sb[:, k * I + i * P: k * I + (i + 1) * P]),
                            (xt_ch[k][:]),
                            start=(k == 0),
                            stop=(k == HK - 1),
                        )
                    nc.scalar.activation(
                        ht_sb[:, i * TB:(i + 1) * TB], g_ps[:], AF.Silu
                    )

                def u_step(i):
                    u_ps = psgu.tile([P, TB], F32, tag="gu", name="u_ps")
                    for k in range(HK):
                        nc.tensor.matmul(
                            u_ps[:],
                            (wu_sb[:, k * I + i * P: k * I + (i + 1) * P]),
                            (xt_ch[k][:]),
                            start=(k == 0),
                            stop=(k == HK - 1),
                        )
                    hsl = ht_sb[:, i * TB:(i + 1) * TB]
                    nc.vector.tensor_tensor(
                        out=hsl, in0=hsl, in1=u_ps[:], op=mybir.AluOpType.mult
                    )

                if b == 0:
                    # k-outer in groups of 4 i-chunks (borrowing psy banks):
                    # each 512KB wg k-chunk unlocks 4 matmuls as it lands.
                    for w_sb, is_g in ((wg_sb, True), (wu_sb, False)):
                        for grp in range(IK // 4):
                            ps4 = [
                                psy.tile([P, TB], F32, tag="y",
                                         name=f"b0_{'g' if is_g else 'u'}{grp}_{j}")
                                for j in range(4)
                            ]
                            for k in range(HK):
                                for j in range(4):
                                    i = grp * 4 + j
                                    nc.tensor.matmul(
                                        ps4[j][:],
                                        (w_sb[:, k * I + i * P:
                                              k * I + (i + 1) * P]),
                                        (xt_ch[k][:]),
                                        start=(k == 0),
                                        stop=(k == HK - 1),
                                    )
                            for j in range(4):
                                i = grp * 4 + j
                                hsl = ht_sb[:, i * TB:(i + 1) * TB]
                                if is_g:
                                    nc.scalar.activation(hsl, ps4[j][:], AF.Silu)
                                else:
                                    nc.vector.tensor_tensor(
                                        out=hsl, in0=hsl, in1=ps4[j][:],
                                        op=mybir.AluOpType.mult,
                                    )
                else:
                    for i in range(IK):
                        g_step(i)
                        u_step(i)
                        # router partials ride mid-stage-1: every ACT-tick
                        # dependency from the previous cycle has drained by
                        # here, so the column-tiled matmuls issue stall-free
                        if i == IK - 2 and b + 1 < NB:
                            p4_next = router_partials(xtq[0])

                # ---- stage 2: Y[m] [128tok, 512h] = hT^T @ Wd (resident),
                # m-OUTER: both h-halves of one token subtile accumulate
                # together (one ht LDWEIGHTS feeds 2 matmuls), and each m's
                # evict+DMA overlaps the next m's 3.5us of matmuls, so no
                # PSUM slot is ever reused without a full group of PE work
                # in between. The router collapse rides between m0 and m1.
                if b == 0 and b + 1 < NB:
                    p4_next = router_partials(xtq[0])
                for m in range(NM):
                    if m == 1 and b + 1 < NB:
                        w_next = router_finish(p4_next)
                    y2 = [
                        psy.tile([P, 512], F32, tag="y", name=f"y{m}_{h}")
                        for h in range(NH)
                    ]
                    for i in range(IK):
                        st = ht_sb[:, i * TB + m * P: i * TB + (m + 1) * P]
                        for h in range(NH):
                            nc.tensor.matmul(
                                y2[h][:],
                                (st),
                                (wd_sb[:, i * H + h * 512:
                                       i * H + (h + 1) * 512]),
                                start=(i == 0),
                                stop=(i == IK - 1),
                            )
                    for h in range(NH):
                        y_sb = evp.tile([P, 512], DT16, tag="ev",
                                        name=f"yev{m}_{h}")
                        nc.vector.tensor_scalar_mul(
                            y_sb[:], y2[h][:], w_tiles[:, m:m + 1]
                        )
                        nc.sync.dma_start(
                            out=y_d[b * TB + m * P: b * TB + (m + 1) * P,
                                    h * 512:(h + 1) * 512],
                            in_=y_sb[:],
                        )

    nc.compile()
    return nc


def kernel(x, W_router, W_gate, W_up, W_down):
    global LAST_EXEC_NS
    if "nc" not in _CACHE:
        _CACHE["nc"] = _build()
    nc = _CACHE["nc"]

    bf16 = np.float16
    x_bf = np.asarray(x, dtype=np.float32).reshape(N, H).astype(bf16)
    xt = np.ascontiguousarray(x_bf.T)
    # repack router weights into the SBUF layout [128, HK*E]: row p holds
    # chunk k's rows (k*128+p) side by side -> plain contiguous DMA on device
    wr = np.ascontiguousarray(
        np.asarray(W_router, dtype=np.float32)
        .reshape(HK, P, E).transpose(1, 0, 2).reshape(P, HK * E)
    ).astype(bf16)
    eye = np.eye(E, dtype=np.float32)
    in_maps = []
    for e in range(E):
        # [ones | one-hot(e)] replicated across the 128 partitions, plus
        # the column-group sum selector s4[p, :] = one-hot(p % 32) for
        # p % 32 < 8 (zero rows elsewhere)
        s4 = ((np.arange(P) % 32)[:, None] == np.arange(E)[None, :])
        sel = np.concatenate([
            np.tile(np.concatenate([np.ones(E, np.float32), eye[e]]), (P, 1)),
            s4.astype(np.float32),
        ], axis=1)
        in_maps.append({
            "xt": xt,
            "wg": np.ascontiguousarray(W_gate[e]).astype(bf16),
            "wu": np.ascontiguousarray(W_up[e]).astype(bf16),
            "wd": np.ascontiguousarray(W_down[e]).astype(bf16),
            "wr": wr,
            "sel": np.ascontiguousarray(sel).astype(bf16),
        })

    res = run_bass_kernel_spmd(nc, in_maps, list(range(E)), trace=TRACE)
    LAST_EXEC_NS = res.exec_time_ns

    acc = np.zeros((N, H), dtype=np.float64)
    for r in res.results:
        acc += r["y"]
    return acc.astype(np.float32).reshape(x.shape[0], x.shape[1], H)


# revision 37
# speedup vs baseline: 1.0024x; 1.0024x over previous
"""MoE layer (dense all-experts SwiGLU + router-weighted sum) on 8 TRN2 cores.

Expert-parallel: core e holds expert e's weights (E=8). Every core sees the
full token stream x (shipped pre-transposed as xT [H, N]) and computes
  y_e = softmax(x @ W_router)[:, e] * ((silu(x@Wg_e) * (x@Wu_e)) @ Wd_e)
The host sums the 8 per-expert outputs in float64.

All matmul operands are fp16: measured on TRN2, fp16 matmuls stream at the
full 2.4 GHz PE clock (216ns per 512-col matmul) while fp32r runs ~2.2 GHz
(233ns) and bf16 only ~2.0 GHz (259ns). fp16 quantization costs ~6e-4 rel
err vs the 2e-2 gate. PSUM accumulation stays fp32. 2-byte weights also fit
ALL of Wg/Wu/Wd resident in SBUF (96KB/partition), so there is no steady-
state weight streaming at all: per block only xt in (1MB, sync HWDGE ring,
3 blocks deep) and y out (1MB fp16, same ring, issued after the xt
prefetch; the last block is m-outer so the tail drains in ~2us).

Per-core program, per 512-token block:
  router: logits via 2 waves of 4 COLUMN-TILED matmuls (col group j streams
          k-chunks j/j+4 concurrently, ~0.45us for what 8 serial matmuls did
          in 1.7us), partials collapsed by one full-K matmul against a
          host-built group-sum selector; Exp on ACT, DVE 32x32 block-
          transposes to token-partition layout, then numer/denom as DVE
          free-dim reductions against [ones | one-hot(e)]; w = num/den.
          Partials issue MID-STAGE-1 (after i=14's u-step), where every
          ACT-tick dependency from the previous cycle has drained - this
          placement removed the last per-block PE stall; the collapse runs
          after stage-2's first m-group, copies long done.
  stage1: G/U [128i, 512tok] = Wg/Wu_chunk^T @ xT_chunk (8 K-chunks into
          PSUM), hT[i] = silu(G)*U -> SBUF fp16 ([I, tok] layout).
  stage2: Y[m] [128tok, 512h] accumulates 16 i-chunks, hT stationary and
          resident Wd slices moving; evict = DVE multiply by w.

Block 0 runs stage 1 k-OUTER in groups of 4 i-chunks (borrowing psy PSUM
banks) so compute chases the 256KB Wg half-chunk DMAs as they land instead
of waiting for all of Wg. All DMA rides the sync HWDGE ring: it wakes
faster than SWDGE (~10.5us vs ~13us to first matmul) and no other queue's
semaphore set gets allocated.

Scheduling notes (hard-won):
  - tile-pool slot allocation order must match consumption order or the
    schedule deadlocks; 24 xt chunk tiles (3 blocks) are live at once.
  - stage 2 is m-OUTER: both h-halves of a token subtile accumulate
    together, so each m's evict+DMA overlaps the next m's matmuls.
  - psgu=3/psy=5 PSUM split: 3 g/u banks decouple the g-start matmuls
    from silu completion latency; router partials share psy's rotation.
  - never read uninitialized SBUF (the exp tile is memset before the
    transposes): it crashes the device, silently.
"""
import numpy as np

import concourse.bass as bass
import concourse.mybir as mybir
import concourse.tile as tile
from concourse import bacc
from concourse.bass_utils import run_bass_kernel_spmd

P = 128
H, I, E = 1024, 2048, 8
N = 8192  # tokens = 4 * 2048
HK = H // P   # 8 contraction chunks over H
IK = I // P   # 16 chunks over I
TB = 512      # token block
NB = N // TB  # 16 blocks
NM = TB // P  # 4 token subtiles per block
NH = H // 512  # 2 output column halves
PF = 3        # xt prefetch depth (blocks)

F32 = mybir.dt.float32
DT16 = mybir.dt.float16
AF = mybir.ActivationFunctionType

# set by a driver (test.py) to profile; harness path keeps defaults
TRACE = False
LAST_EXEC_NS = None

_CACHE = {}


def _build():
    nc = bacc.Bacc("TRN2", target_bir_lowering=False, debug=False)

    xt_d = nc.dram_tensor("xt", [H, N], DT16, kind="ExternalInput").ap()
    wg_d = nc.dram_tensor("wg", [H, I], DT16, kind="ExternalInput").ap()
    wu_d = nc.dram_tensor("wu", [H, I], DT16, kind="ExternalInput").ap()
    wd_d = nc.dram_tensor("wd", [I, H], DT16, kind="ExternalInput").ap()
    wr_d = nc.dram_tensor("wr", [P, HK * E], DT16, kind="ExternalInput").ap()
    sel_d = nc.dram_tensor("sel", [P, 3 * E], DT16, kind="ExternalInput").ap()
    y_d = nc.dram_tensor("y", [N, H], DT16, kind="ExternalOutput").ap()

    with tile.TileContext(nc) as tc:
        with (
            tc.tile_pool(name="const", bufs=1) as const,
            tc.tile_pool(name="xtp", bufs=PF * HK) as xtp,
            tc.tile_pool(name="htp", bufs=1) as htp,
            tc.tile_pool(name="evp", bufs=8) as evp,
            tc.tile_pool(name="rtp", bufs=2) as rtp,
            tc.tile_pool(name="wp", bufs=2) as wp,
            tc.tile_pool(name="psgu", bufs=3, space="PSUM") as psgu,
            tc.tile_pool(name="psy", bufs=5, space="PSUM") as psy,
        ):
            # resident weights: [128, HK*I] with chunk k at cols [k*I, (k+1)*I)
            wg_sb = const.tile([P, HK * I], DT16)
            wu_sb = const.tile([P, HK * I], DT16)
            # wd resident: [128, IK*H] with i-chunk at cols [i*H, (i+1)*H)
            wd_sb = const.tile([P, IK * H], DT16)
            wr_sb = const.tile([P, HK * E], DT16)
            sel_sb = const.tile([P, 3 * E], DT16)

            def load_xt(b, eng=None):
                eng = eng or nc.sync
                tok = slice(b * TB, (b + 1) * TB)
                chunks = []
                for k in range(HK):
                    ch = xtp.tile([P, TB], DT16, tag="xt", name=f"xt{b}_{k}")
                    eng.dma_start(
                        out=ch[:], in_=xt_d[k * P:(k + 1) * P, tok]
                    )
                    chunks.append(ch)
                return chunks

            # ---- prologue DMAs in consumption order, all on the sync
            # ring (it wakes faster than SWDGE: first matmul 10.5us vs
            # 13.3us measured; and never touching SWDGE also skips its
            # semaphore set in the teardown).
            IH = I // 2
            nc.sync.dma_start(out=wr_sb[:], in_=wr_d[:])
            xt_next = load_xt(0)
            nc.sync.dma_start(out=sel_sb[:], in_=sel_d[:])
            # wg streamed as 256KB half-chunks, half-A (i 0-7) of every k
            # first: block-0's k-outer groups consume them as they land.
            for half in range(2):
                for k in range(HK):
                    nc.sync.dma_start(
                        out=wg_sb[:, k * I + half * IH: k * I + (half + 1) * IH],
                        in_=wg_d[k * P:(k + 1) * P, half * IH:(half + 1) * IH],
                    )
            xt_pre1 = load_xt(1)
            for half in range(2):
                for k in range(HK):
                    nc.sync.dma_start(
                        out=wu_sb[:, k * I + half * IH: k * I + (half + 1) * IH],
                        in_=wu_d[k * P:(k + 1) * P, half * IH:(half + 1) * IH],
                    )
            for j in range(IK // 2):  # wd as 512KB 2-chunk batches
                rows = slice(2 * j * P, (2 * j + 2) * P)
                nc.sync.dma_start(
                    out=wd_sb[:, 2 * j * H:(2 * j + 2) * H].rearrange(
                        "p (j c) -> p j c", j=2),
                    in_=wd_d[rows, :].rearrange("(j p) c -> p j c", p=P),
                )

            def router_partials(xt_ch):
                # w[tok] = softmax(logits)[:, e] for one block. Only the 8
                # lt matmuls touch the PE: exp'd logits move to token-
                # partition layout via DVE 32x32 block transposes, and
                # numer/denom come from DVE free-dim accumulations against
                # selbc ([ones | one-hot(e)] replicated per partition).
                # 4-way column-tiled logits: col group j streams k-chunks
                # j and j+4 concurrently into partitions 32j..32j+7 of one
                # PSUM bank (~2 wave-times instead of 8 serial matmuls).
                # Only the first matmul clears the bank's has_written bits;
                # the other groups' first writes land on cleared bits and
                # overwrite, wave 2 accumulates.
                p4s = [
                    psy.tile([P, TB], F32, tag="y", name=f"lt4_{j}")
                    for j in range(4)
                ]
                for w in range(2):
                    for j in range(4):
                        k = 4 * w + j
                        nc.tensor.matmul(
                            p4s[j][32 * j:32 * j + E, :],
                            (wr_sb[:, k * E:(k + 1) * E]),
                            (xt_ch[k][:]),
                            start=(w == 0),
                            stop=(w == 1),
                            tile_position=(0, 32 * j),
                            skip_group_check=True,
                        )
                # collapse the 4 partition-group partials with one full-K
                # matmul against the group-sum selector. p4 is memset first:
                # the PE streams all 128 partitions and must never read
                # uninitialized SBUF (zero rows also zero out the garbage
                # via the selector's zero rows).
                p4 = rtp.tile([P, TB], DT16, tag="p4", name="p4")
                nc.vector.memset(p4[:], 0.0)
                for j in range(4):
                    nc.scalar.activation(
                        p4[32 * j:32 * j + E, :], p4s[j][32 * j:32 * j + E, :],
                        AF.Copy,
                    )
                return p4

            def router_finish(p4):
                # runs between the h-sweeps: by then the partial copies have
                # drained, so the sum-matmul issues without a PE stall
                lt = psgu.tile([E, TB], F32, tag="gu", name="lt")
                nc.tensor.matmul(
                    lt[:], (sel_sb[:, 2 * E:3 * E]), (p4[:]),
                    start=True, stop=True,
                )
                # zero all 32 rows first (partition base must be 32-
                # aligned, and the transposes must not read uninitialized
                # SBUF), then exp overwrites rows 0-7.
                exp_sb = rtp.tile([32, TB], DT16, tag="exp", name="exp_sb")
                nc.vector.memset(exp_sb[:], 0.0)
                nc.scalar.activation(exp_sb[0:E, :], lt[:], AF.Exp)
                wv = wp.tile([P, NM], F32, tag="wv", name="wv")
                for m in range(NM):
                    et = rtp.tile([P, 32], DT16, tag="et", name="et", bufs=4)
                    for j in range(4):
                        nc.vector.transpose(
                            out=et[j * 32:(j + 1) * 32, 0:32],
                            in_=exp_sb[0:32, m * P + j * 32:
                                       m * P + (j + 1) * 32],
                        )
                    junk = wp.tile([P, E], F32, tag="junk", name="junk")
                    den = wp.tile([P, 1], F32, tag="den", name="den")
                    nc.vector.scalar_tensor_tensor(
                        out=junk[:], in0=et[:, 0:E], scalar=1.0,
                        in1=sel_sb[:, 0:E], op0=mybir.AluOpType.mult,
                        op1=mybir.AluOpType.mult, accum_out=den[:],
                    )
                    num = wp.tile([P, 1], F32, tag="num", name="num")
                    nc.vector.scalar_tensor_tensor(
                        out=junk[:], in0=et[:, 0:E], scalar=1.0,
                        in1=sel_sb[:, E:2 * E], op0=mybir.AluOpType.mult,
                        op1=mybir.AluOpType.mult, accum_out=num[:],
                    )
                    rec = wp.tile([P, 1], F32, tag="rec", name="rec")
                    nc.vector.reciprocal(rec[:], den[:])
                    nc.vector.tensor_tensor(
                        out=wv[:, m:m + 1], in0=num[:], in1=rec[:],
                        op=mybir.AluOpType.mult,
                    )
                return wv

            xtq = [xt_next, xt_pre1]  # blocks b, b+1 (already issued)
            w_next = router_finish(router_partials(xtq[0]))
            for b in range(NB):
                xt_ch = xtq.pop(0)
                w_tiles = w_next
                if b + 2 < NB:
                    xtq.append(load_xt(b + 2))

                # ---- stage 1: hT[i] = silu(G)*U, [I-chunk, tok] layout
                ht_sb = htp.tile([P, IK * TB], DT16, tag="ht")

                def g_step(i):
                    g_ps = psgu.tile([P, TB], F32, tag="gu", name="g_ps")
                    for k in range(HK):
                        nc.tensor.matmul(
                            g_ps[:],
                            (wg_sb[:, k * I + i * P: k * I + (i + 1) * P]),
                            (xt_ch[k][:]),
                            start=(k == 0),
                            stop=(k == HK - 1),
                        )
                    nc.scalar.activation(
                        ht_sb[:, i * TB:(i + 1) * TB], g_ps[:], AF.Silu
                    )

                def u_step(i):
                    u_ps = psgu.tile([P, TB], F32, tag="gu", name="u_ps")
                    for k in range(HK):
                        nc.tensor.matmul(
                            u_ps[:],
                            (wu_sb[:, k * I + i * P: k * I + (i + 1) * P]),
                            (xt_ch[k][:]),
                            start=(k == 0),
                            stop=(k == HK - 1),
                        )
                    hsl = ht_sb[:, i * TB:(i + 1) * TB]
                    nc.vector.tensor_tensor(
                        out=hsl, in0=hsl, in1=u_ps[:], op=mybir.AluOpType.mult
                    )

                if b == 0:
                    # k-outer in groups of 4 i-chunks (borrowing psy banks):
                    # each 512KB wg k-chunk unlocks 4 matmuls as it lands.
                    for w_sb, is_g in ((wg_sb, True), (wu_sb, False)):
                        for grp in range(IK // 4):
                            ps4 = [
                                psy.tile([P, TB], F32, tag="y",
                                         name=f"b0_{'g' if is_g else 'u'}{grp}_{j}")
                                for j in range(4)
                            ]
                            for k in range(HK):
                                for j in range(4):
                                    i = grp * 4 + j
                                    nc.tensor.matmul(
                                        ps4[j][:],
                                        (w_sb[:, k * I + i * P:
                                              k * I + (i + 1) * P]),
                                        (xt_ch[k][:]),
                                        start=(k == 0),
                                        stop=(k == HK - 1),
                                    )
                            for j in range(4):
                                i = grp * 4 + j
                                hsl = ht_sb[:, i * TB:(i + 1) * TB]
                                if is_g:
                                    nc.scalar.activation(hsl, ps4[j][:], AF.Silu)
                                else:
                                    nc.vector.tensor_tensor(
                                        out=hsl, in0=hsl, in1=ps4[j][:],
                                        op=mybir.AluOpType.mult,
                                    )
                else:
                    for i in range(IK):
                        g_step(i)
                        u_step(i)
                        # router partials ride mid-stage-1: every ACT-tick
                        # dependency from the previous cycle has drained by
                        # here, so the column-tiled matmuls issue stall-free
                        if i == IK - 2 and b + 1 < NB:
                            p4_next = router_partials(xtq[0])

                # ---- stage 2: Y[m] [128tok, 512h] = hT^T @ Wd (resident),
                # m-OUTER: both h-halves of one token subtile accumulate
                # together (one ht LDWEIGHTS feeds 2 matmuls), and each m's
                # evict+DMA overlaps the next m's 3.5us of matmuls, so no
                # PSUM slot is ever reused without a full group of PE work
                # in between. The router collapse rides between m0 and m1.
                if b == 0 and b + 1 < NB:
                    p4_next = router_partials(xtq[0])
                for m in range(NM):
                    if m == 1 and b + 1 < NB:
                        w_next = router_finish(p4_next)
                    y2 = [
                        psy.tile([P, 512], F32, tag="y", name=f"y{m}_{h}")
                        for h in range(NH)
                    ]
                    for i in range(IK):
                        st = ht_sb[:, i * TB + m * P: i * TB + (m + 1) * P]
                        for h in range(NH):
                            nc.tensor.matmul(
                                y2[h][:],
                                (st),
                                (wd_sb[:, i * H + h * 512:
                                       i * H + (h + 1) * 512]),
                                start=(i == 0),
                                stop=(i == IK - 1),
                            )
                    # both halves evict in parallel (DVE h0, ACT h1) into
                    # one tile -> a single full-row 256KB DMA per m-group
                    y_sb = evp.tile([P, H], DT16, tag="ev", name=f"yev{m}")
                    nc.vector.tensor_scalar_mul(
                        y_sb[:, 0:512], y2[0][:], w_tiles[:, m:m + 1]
                    )
                    nc.scalar.activation(
                        y_sb[:, 512:H], y2[1][:], AF.Copy,
                        scale=w_tiles[:, m:m + 1],
                    )
                    nc.sync.dma_start(
                        out=y_d[b * TB + m * P: b * TB + (m + 1) * P, :],
                        in_=y_sb[:],
                    )

    nc.compile()
    return nc


def kernel(x, W_router, W_gate, W_up, W_down):
    global LAST_EXEC_NS
    if "nc" not in _CACHE:
        _CACHE["nc"] = _build()
    nc = _CACHE["nc"]

    bf16 = np.float16
    x_bf = np.asarray(x, dtype=np.float32).reshape(N, H).astype(bf16)
    xt = np.ascontiguousarray(x_bf.T)
    # repack router weights into the SBUF layout [128, HK*E]: row p holds
    # chunk k's rows (k*128+p) side by side -> plain contiguous DMA on device
    wr = np.ascontiguousarray(
        np.asarray(W_router, dtype=np.float32)
        .reshape(HK, P, E).transpose(1, 0, 2).reshape(P, HK * E)
    ).astype(bf16)
    eye = np.eye(E, dtype=np.float32)
    in_maps = []
    for e in range(E):
        # [ones | one-hot(e)] replicated across the 128 partitions, plus
        # the column-group sum selector s4[p, :] = one-hot(p % 32) for
        # p % 32 < 8 (zero rows elsewhere)
        s4 = ((np.arange(P) % 32)[:, None] == np.arange(E)[None, :])
        sel = np.concatenate([
            np.tile(np.concatenate([np.ones(E, np.float32), eye[e]]), (P, 1)),
            s4.astype(np.float32),
        ], axis=1)
        in_maps.append({
            "xt": xt,
            "wg": np.ascontiguousarray(W_gate[e]).astype(bf16),
            "wu": np.ascontiguousarray(W_up[e]).astype(bf16),
            "wd": np.ascontiguousarray(W_down[e]).astype(bf16),
            "wr": wr,
            "sel": np.ascontiguousarray(sel).astype(bf16),
        })

    res = run_bass_kernel_spmd(nc, in_maps, list(range(E)), trace=TRACE)
    LAST_EXEC_NS = res.exec_time_ns

    acc = np.zeros((N, H), dtype=np.float64)
    for r in res.results:
        acc += r["y"]
    return acc.astype(np.float32).reshape(x.shape[0], x.shape[1], H)
